# revision 1
# baseline (speedup 1.0000x reference)
"""DAM (Deep Attention Matching over bundles) Trainium2 kernel.

Reference computation (per bundle b over its <=L items):
    h_u = emb_u[x_u]                                  [B, d]
    h_b = emb_b[x_b]                                  [B, d]
    scores[b, l] = h_u[b] . A[items[b, l]]            (masked)
    w = softmax_l(scores)                             [B, L]
    h_x[b] = sum_l w[b, l] * emb_i[items[b, l]]       [B, d]
    h = concat([h_u, h_b + h_x])                      [B, 2d]
    out = leaky(leaky(h @ W1.T + b1) @ W2.T + b2) @ out_w.T + out_b

Strategy: data-parallel over B across 8 cores (1024 bundles each). The memory
traffic is dominated by gathering the per-item rows of emb_i and A. The two
tables are interleaved on the host into one [N_ITEM, 2d] table so each item
needs ONE 1KB gather descriptor. Masked slots use an out-of-range sentinel
index so the DGE skips them (no HBM read). One indirect DMA per 32-bundle
group gathers 4096 rows ([128, 32] indices -> flat [128, 32*256] dest, which
amortizes the Q7 SWDGE emission to ~1.7ns/row).

On-chip layout: items-on-partitions, one 256-wide slot per bundle. Per bundle:
  - PE broadcasts h_u[j] to all partitions (rank-1 matmul with ones),
  - DVE computes scores via fused tensor_tensor_reduce (mul + add-reduce
    over d) -> scores column [128, 1],
  - softmax runs on the 32x128 transpose (DVE 32x32 stream transposes),
  - PE computes h_x[j].T = item_e_j.T @ w_j as a K=128,N=1 matmul, placing
    h_x.T columns directly in the transposed layout the FC stack needs.
The FC layers run as PE matmuls on [256, 32] activations.
"""

import os
import sys

import numpy as np

sys.path.insert(0, "/opt/trn_rl_repo")

import concourse.bacc as bacc
import concourse.bass as bass
import concourse.tile as tile
from concourse import mybir
from concourse.bass import IndirectOffsetOnAxis
from concourse.bass_utils import run_bass_kernel_spmd

# Problem dims (hardcoded per spec)
N_USER, N_ITEM, N_BUNDLE = 200000, 100000, 50000
D = 128            # embed dim
D2 = 2 * D         # fc width
B = 8192           # total bundles
L = 100            # max items/bundle
LP = 128           # padded item-partition dim
NCORES = 8
BC = B // NCORES   # bundles per core = 1024
NB = 32            # bundles per group
NG = BC // NB      # groups per core = 32
GBUFS = 3          # gather tile buffering depth
SENTINEL = N_ITEM  # out-of-range index -> DGE skips the row

FP = mybir.dt.float32
I32 = mybir.dt.int32

_cache = {}
last_results = None


def _build_program(alpha: float, out_b0: float):
    """Build the per-core Bass/Tile program (identical on all 8 cores)."""
    nc = bacc.Bacc("TRN2", target_bir_lowering=False, debug=False,
                   num_devices=NCORES)

    tab = nc.dram_tensor("tab", [N_ITEM, D2], FP, kind="ExternalInput")
    embu = nc.dram_tensor("embu", [N_USER, D], FP, kind="ExternalInput")
    embb = nc.dram_tensor("embb", [N_BUNDLE, D], FP, kind="ExternalInput")
    idxt = nc.dram_tensor("idxt", [LP, BC], I32, kind="ExternalInput")
    mbias = nc.dram_tensor("mbias", [NB, NG * LP], FP, kind="ExternalInput")
    xuf = nc.dram_tensor("xuf", [1, BC], I32, kind="ExternalInput")
    xupm = nc.dram_tensor("xupm", [NB, NG], I32, kind="ExternalInput")
    xbpm = nc.dram_tensor("xbpm", [NB, NG], I32, kind="ExternalInput")
    w1pm = nc.dram_tensor("w1pm", [D, 2, D2], FP, kind="ExternalInput")
    w2pm = nc.dram_tensor("w2pm", [D, 2, D2], FP, kind="ExternalInput")
    b1pm = nc.dram_tensor("b1pm", [D, 2], FP, kind="ExternalInput")
    b2pm = nc.dram_tensor("b2pm", [D, 2], FP, kind="ExternalInput")
    owpm = nc.dram_tensor("owpm", [D, 2], FP, kind="ExternalInput")
    out = nc.dram_tensor("out", [1, BC], FP, kind="ExternalOutput")

    mult = mybir.AluOpType.mult
    add = mybir.AluOpType.add
    amax = mybir.AluOpType.max

    with tile.TileContext(nc) as tc:
        with (
            tc.tile_pool(name="const", bufs=1) as constp,
            tc.tile_pool(name="gather", bufs=GBUFS) as gatherp,
            tc.tile_pool(name="prod", bufs=2) as prodp,
            tc.tile_pool(name="sm", bufs=2) as smp,
            tc.tile_pool(name="act", bufs=2) as actp,
            tc.tile_pool(name="hu", bufs=2) as hup,
            tc.tile_pool(name="pbc", bufs=2, space="PSUM") as pbcp,
            tc.tile_pool(name="phx", bufs=2, space="PSUM") as phxp,
            tc.tile_pool(name="pfc", bufs=2, space="PSUM") as pfcp,
        ):
            # ---- one-time loads (HWDGE) ----
            idxt_sb = constp.tile([LP, BC], I32)
            nc.sync.dma_start(idxt_sb[:], idxt.ap())
            xuf_sb = constp.tile([1, BC], I32)
            nc.sync.dma_start(xuf_sb[:], xuf.ap())
            mb_sb = constp.tile([NB, NG * LP], FP)
            nc.sync.dma_start(mb_sb[:], mbias.ap())
            xupm_sb = constp.tile([NB, NG], I32)
            nc.sync.dma_start(xupm_sb[:], xupm.ap())
            xbpm_sb = constp.tile([NB, NG], I32)
            nc.sync.dma_start(xbpm_sb[:], xbpm.ap())
            w1_sb = constp.tile([D, 2, D2], FP)
            nc.sync.dma_start(w1_sb[:], w1pm.ap())
            w2_sb = constp.tile([D, 2, D2], FP)
            nc.sync.dma_start(w2_sb[:], w2pm.ap())
            b1_sb = constp.tile([D, 2], FP)
            nc.sync.dma_start(b1_sb[:], b1pm.ap())
            b2_sb = constp.tile([D, 2], FP)
            nc.sync.dma_start(b2_sb[:], b2pm.ap())
            ow_sb = constp.tile([D, 2], FP)
            nc.sync.dma_start(ow_sb[:], owpm.ap())
            ones_sb = constp.tile([1, D], FP)
            nc.vector.memset(ones_sb[:], 1.0)
            out_acc = constp.tile([1, BC], FP)

            # all h_u / h_b rows, one row per (partition=in-group idx,
            # block=group): hur_all[p, g*D:(g+1)*D] = emb_u[x_u[g*NB+p]]
            # (the DGE fetches exactly one index per partition per call)
            hur_all = constp.tile([NB, NG * D], FP)
            hbr_all = constp.tile([NB, NG * D], FP)
            for g0 in range(NG):
                d0 = slice(g0 * D, (g0 + 1) * D)
                nc.gpsimd.indirect_dma_start(
                    out=hur_all[:, d0], out_offset=None, in_=embu.ap(),
                    in_offset=IndirectOffsetOnAxis(
                        ap=xupm_sb[:, g0:g0 + 1], axis=0),
                )
                nc.gpsimd.indirect_dma_start(
                    out=hbr_all[:, d0], out_offset=None, in_=embb.ap(),
                    in_offset=IndirectOffsetOnAxis(
                        ap=xbpm_sb[:, g0:g0 + 1], axis=0),
                )

            for g in range(NG):
                gsl = slice(g * NB, (g + 1) * NB)
                dsl = slice(g * D, (g + 1) * D)

                # ---- gather item rows: one [128,1]-index call per bundle ----
                gt = gatherp.tile([LP, NB * D2], FP)
                if g < GBUFS:
                    # first use of each buffer: clear stale/NaN SBUF so
                    # skipped (masked) rows stay finite
                    nc.vector.memset(gt[:], 0.0)
                for j in range(NB):
                    nc.gpsimd.indirect_dma_start(
                        out=gt[:, j * D2:(j + 1) * D2],
                        out_offset=None,
                        in_=tab.ap(),
                        in_offset=IndirectOffsetOnAxis(
                            ap=idxt_sb[:, g * NB + j:g * NB + j + 1], axis=0),
                        bounds_check=N_ITEM - 1,
                        oob_is_err=False,
                    )

                # ---- flat copy of the group's h_u rows onto partition 0
                # (PE rhs must start at partition 0/32/64, so collapse the
                # 32 per-partition rows into one 4KB row via SBUF->SBUF DMA)
                huf = hup.tile([1, NB * D], FP, tag="huf")
                nc.sync.dma_start(huf[:], hur_all[:, dsl])

                # ---- scores: per-bundle dot(A-half, h_u[j]) ----
                scores_g = smp.tile([LP, NB], FP, tag="scores")
                for q in range(NB // 4):
                    pbc = pbcp.tile([LP, 4 * D], FP, space="PSUM")
                    nc.tensor.matmul(
                        pbc[:], lhsT=ones_sb[:],
                        rhs=huf[:, q * 4 * D:(q + 1) * 4 * D],
                        start=True, stop=True,
                    )
                    for jj in range(4):
                        j = q * 4 + jj
                        prod = prodp.tile([LP, D], FP, tag="prod")
                        nc.vector.tensor_tensor(
                            out=prod[:],
                            in0=gt[:, j * D2 + D:(j + 1) * D2],
                            in1=pbc[:, jj * D:(jj + 1) * D],
                            op=mult,
                        )
                        nc.vector.reduce_sum(
                            out=scores_g[:, j:j + 1], in_=prod[:],
                            axis=mybir.AxisListType.X,
                        )

                # ---- softmax over items (transpose to bundles-on-partitions) ----
                scT = smp.tile([NB, LP], FP, tag="scT")
                for r in range(4):
                    nc.vector.transpose(
                        out=scT[0:32, r * 32:(r + 1) * 32],
                        in_=scores_g[r * 32:(r + 1) * 32, 0:32],
                    )
                sadj = smp.tile([NB, LP], FP, tag="sadj")
                mx = smp.tile([NB, 1], FP, tag="mx")
                nc.vector.tensor_tensor(
                    out=sadj[:],
                    in0=scT[:],
                    in1=mb_sb[:, g * LP:(g + 1) * LP],
                    op=add,
                )
                nc.vector.reduce_max(
                    out=mx[:], in_=sadj[:], axis=mybir.AxisListType.X,
                )
                nmx = smp.tile([NB, 1], FP, tag="nmx")
                nc.scalar.mul(nmx[:], mx[:], -1.0)
                ex = smp.tile([NB, LP], FP, tag="ex")
                zs = smp.tile([NB, 1], FP, tag="zs")
                nc.scalar.activation(
                    ex[:], sadj[:], mybir.ActivationFunctionType.Exp,
                    bias=nmx[:], scale=1.0, accum_out=zs[:],
                )
                rz = smp.tile([NB, 1], FP, tag="rz")
                nc.vector.reciprocal(rz[:], zs[:])
                wT = smp.tile([NB, LP], FP, tag="wT")
                nc.vector.tensor_scalar_mul(wT[:], ex[:], rz[:])
                w_g = smp.tile([LP, NB], FP, tag="w_g")
                for r in range(4):
                    nc.vector.transpose(
                        out=w_g[r * 32:(r + 1) * 32, 0:32],
                        in_=wT[0:32, r * 32:(r + 1) * 32],
                    )

                # ---- h_x.T columns via PE: item_e_j.T @ w_j ----
                phx = phxp.tile([D, NB], FP, space="PSUM")
                for j in range(NB):
                    nc.tensor.matmul(
                        phx[:, j:j + 1],
                        lhsT=gt[:, j * D2:j * D2 + D],
                        rhs=w_g[:, j:j + 1],
                        start=True, stop=True,
                    )

                # ---- transposes of h_u, h_b into [d, nb] ----
                huT = actp.tile([D, NB], FP, tag="huT")
                hbT = actp.tile([D, NB], FP, tag="hbT")
                for r in range(4):
                    nc.vector.transpose(
                        out=huT[r * 32:(r + 1) * 32, 0:32],
                        in_=hur_all[0:32, g * D + r * 32:g * D + (r + 1) * 32],
                    )
                    nc.vector.transpose(
                        out=hbT[r * 32:(r + 1) * 32, 0:32],
                        in_=hbr_all[0:32, g * D + r * 32:g * D + (r + 1) * 32],
                    )
                hbot = actp.tile([D, NB], FP, tag="hbot")
                nc.vector.tensor_add(hbot[:], phx[:], hbT[:])

                # ---- FC stack on [256, 32] activations ----
                def fc(w_sb, b_sb, rhs0, rhs1, tag):
                    outs = []
                    for mo in range(2):
                        pfc = pfcp.tile([D, NB], FP, space="PSUM", tag="pfc")
                        msl = slice(mo * D, (mo + 1) * D)
                        nc.tensor.matmul(pfc[:], lhsT=w_sb[:, 0, msl],
                                         rhs=rhs0[:], start=True, stop=False)
                        nc.tensor.matmul(pfc[:], lhsT=w_sb[:, 1, msl],
                                         rhs=rhs1[:], start=False, stop=True)
                        # leaky relu: max(x + b, alpha * (x + b))
                        xb = actp.tile([D, NB], FP, tag=f"{tag}xb{mo}")
                        nc.vector.tensor_scalar_add(xb[:], pfc[:],
                                                    b_sb[:, mo:mo + 1])
                        xs = actp.tile([D, NB], FP, tag=f"{tag}xs{mo}")
                        nc.vector.tensor_scalar_mul(xs[:], xb[:], alpha)
                        ao = actp.tile([D, NB], FP, tag=f"{tag}a{mo}")
                        nc.vector.tensor_tensor(out=ao[:], in0=xb[:],
                                                in1=xs[:], op=amax)
                        outs.append(ao)
                    return outs

                a1 = fc(w1_sb, b1_sb, huT, hbot, "f1")
                a2 = fc(w2_sb, b2_sb, a1[0], a1[1], "f2")

                po = phxp.tile([1, NB], FP, space="PSUM", tag="po")
                nc.tensor.matmul(po[:], lhsT=ow_sb[:, 0:1], rhs=a2[0][:],
                                 start=True, stop=False)
                nc.tensor.matmul(po[:], lhsT=ow_sb[:, 1:2], rhs=a2[1][:],
                                 start=False, stop=True)
                nc.scalar.activation(out_acc[:, gsl], po[:],
                                     mybir.ActivationFunctionType.Copy,
                                     bias=out_b0, scale=1.0)

            nc.sync.dma_start(out.ap(), out_acc[:])

    nc.compile()
    return nc


def _prep_inputs(x_u, x_b, items, mask, emb_u, emb_i, emb_b, A,
                 fc1_w, fc1_b, fc2_w, fc2_b, out_w, out_b):
    """Host-side packing: merged table, transposed/sentineled indices,
    per-core shards."""
    x_u = np.asarray(x_u).astype(np.int32)
    x_b = np.asarray(x_b).astype(np.int32)
    items = np.asarray(items).astype(np.int32)
    mask = np.asarray(mask).astype(bool)
    emb_u = np.ascontiguousarray(np.asarray(emb_u, dtype=np.float32))
    emb_i = np.asarray(emb_i, dtype=np.float32)
    emb_b = np.ascontiguousarray(np.asarray(emb_b, dtype=np.float32))
    A = np.asarray(A, dtype=np.float32)
    fc1_w = np.asarray(fc1_w, dtype=np.float32)
    fc2_w = np.asarray(fc2_w, dtype=np.float32)
    out_w = np.asarray(out_w, dtype=np.float32)
    fc1_b = np.asarray(fc1_b, dtype=np.float32)
    fc2_b = np.asarray(fc2_b, dtype=np.float32)

    tab = np.ascontiguousarray(
        np.concatenate([emb_i, A], axis=1))            # [N_ITEM, 256]

    items_s = np.where(mask, items, SENTINEL)          # [B, L]
    w1pm = np.ascontiguousarray(
        fc1_w.T.reshape(2, D, D2).transpose(1, 0, 2))
    w2pm = np.ascontiguousarray(
        fc2_w.T.reshape(2, D, D2).transpose(1, 0, 2))
    b1pm = np.ascontiguousarray(fc1_b.reshape(2, D).T)
    b2pm = np.ascontiguousarray(fc2_b.reshape(2, D).T)
    owpm = np.ascontiguousarray(out_w.reshape(2, D).T)

    in_maps = []
    for c in range(NCORES):
        bsl = slice(c * BC, (c + 1) * BC)
        it_c = items_s[bsl]                            # [1024, 100]
        idxt = np.full((LP, BC), SENTINEL, np.int32)
        idxt[:L, :] = it_c.T
        mk = mask[bsl]                                 # [1024, 100]
        mb = np.full((BC, LP), -1.0e30, np.float32)
        mb[:, :L][mk] = 0.0
        # [BC, LP] -> [NB, NG*LP] with bundle (g*NB+p) -> partition p, block g
        mbias = np.ascontiguousarray(
            mb.reshape(NG, NB, LP).transpose(1, 0, 2).reshape(NB, NG * LP))
        xu_c = x_u[bsl]
        xb_c = x_b[bsl]
        in_maps.append({
            "tab": tab,
            "embu": emb_u,
            "embb": emb_b,
            "idxt": np.ascontiguousarray(idxt),
            "mbias": mbias,
            "xuf": np.ascontiguousarray(xu_c.reshape(1, BC)),
            "xupm": np.ascontiguousarray(xu_c.reshape(NG, NB).T),
            "xbpm": np.ascontiguousarray(xb_c.reshape(NG, NB).T),
            "w1pm": w1pm,
            "w2pm": w2pm,
            "b1pm": b1pm,
            "b2pm": b2pm,
            "owpm": owpm,
        })
    return in_maps


def kernel(x_u, x_b, items, mask, emb_u, emb_i, emb_b, A,
           fc1_w, fc1_b, fc2_w, fc2_b, out_w, out_b):
    global last_results
    out_b0 = float(np.asarray(out_b, dtype=np.float32).reshape(-1)[0])
    key = ("prog", out_b0)
    if key not in _cache:
        _cache[key] = _build_program(alpha=0.01, out_b0=out_b0)
    nc = _cache[key]

    in_maps = _prep_inputs(x_u, x_b, items, mask, emb_u, emb_i, emb_b, A,
                           fc1_w, fc1_b, fc2_w, fc2_b, out_w, out_b)

    res = run_bass_kernel_spmd(
        nc, in_maps, core_ids=list(range(NCORES)),
        trace=bool(int(os.environ.get("DAM_TRACE", "0"))),
    )
    last_results = res
    outs = [res.results[c]["out"].reshape(BC, 1) for c in range(NCORES)]
    return np.concatenate(outs, axis=0).astype(np.float32)



# revision 11
# speedup vs baseline: 1.0345x; 1.0345x over previous
"""DAM (Deep Attention Matching over bundles) Trainium2 kernel.

Reference computation (per bundle b over its <=L items):
    h_u = emb_u[x_u]                                  [B, d]
    h_b = emb_b[x_b]                                  [B, d]
    scores[b, l] = h_u[b] . A[items[b, l]]            (masked)
    w = softmax_l(scores)                             [B, L]
    h_x[b] = sum_l w[b, l] * emb_i[items[b, l]]       [B, d]
    h = concat([h_u, h_b + h_x])                      [B, 2d]
    out = leaky(leaky(h @ W1.T + b1) @ W2.T + b2) @ out_w.T + out_b

Strategy: data-parallel over B across 8 cores (1024 bundles each). The memory
traffic is dominated by gathering the per-item rows of emb_i and A. The two
tables are interleaved on the host into one [N_ITEM, 2d] table so each item
needs ONE 1KB gather descriptor. Masked slots use an out-of-range sentinel
index so the DGE skips them (no HBM read). One indirect DMA per 32-bundle
group gathers 4096 rows ([128, 32] indices -> flat [128, 32*256] dest, which
amortizes the Q7 SWDGE emission to ~1.7ns/row).

On-chip layout: items-on-partitions, one 256-wide slot per bundle. Per bundle:
  - PE broadcasts h_u[j] to all partitions (rank-1 matmul with ones),
  - DVE computes scores via fused tensor_tensor_reduce (mul + add-reduce
    over d) -> scores column [128, 1],
  - softmax runs on the 32x128 transpose (DVE 32x32 stream transposes),
  - PE computes h_x[j].T = item_e_j.T @ w_j as a K=128,N=1 matmul, placing
    h_x.T columns directly in the transposed layout the FC stack needs.
The FC layers run as PE matmuls on [256, 32] activations.
"""

import os
import sys

import numpy as np
import ml_dtypes

sys.path.insert(0, "/opt/trn_rl_repo")

import concourse.bacc as bacc
import concourse.bass as bass
import concourse.tile as tile
from concourse import mybir
from concourse.bass import IndirectOffsetOnAxis
from concourse.bass_utils import run_bass_kernel_spmd

# Problem dims (hardcoded per spec)
N_USER, N_ITEM, N_BUNDLE = 200000, 100000, 50000
D = 128            # embed dim
D2 = 2 * D         # fc width
B = 8192           # total bundles
L = 100            # max items/bundle
LP = 128           # padded item-partition dim
NCORES = 8
BC = B // NCORES   # bundles per core = 1024
NB = 32            # bundles per group
NG = BC // NB      # groups per core = 32
GBUFS = 3          # gather tile buffering depth
SENTINEL = N_ITEM  # out-of-range index -> DGE skips the row

FP = mybir.dt.float32
BF = mybir.dt.bfloat16
I32 = mybir.dt.int32
BF_NP = ml_dtypes.bfloat16

_cache = {}
last_results = None


def _build_program(alpha: float, out_b0: float):
    """Build the per-core Bass/Tile program (identical on all 8 cores)."""
    nc = bacc.Bacc("TRN2", target_bir_lowering=False, debug=False,
                   num_devices=NCORES)

    tab = nc.dram_tensor("tab", [N_ITEM, D2], BF, kind="ExternalInput")
    embu = nc.dram_tensor("embu", [N_USER, D], FP, kind="ExternalInput")
    embb = nc.dram_tensor("embb", [N_BUNDLE, D], FP, kind="ExternalInput")
    idxt = nc.dram_tensor("idxt", [LP, BC], I32, kind="ExternalInput")
    mbias = nc.dram_tensor("mbias", [NB, NG * LP], FP, kind="ExternalInput")
    xuf = nc.dram_tensor("xuf", [1, BC], I32, kind="ExternalInput")
    xupm = nc.dram_tensor("xupm", [NB, NG], I32, kind="ExternalInput")
    xbpm = nc.dram_tensor("xbpm", [NB, NG], I32, kind="ExternalInput")
    w1pm = nc.dram_tensor("w1pm", [D, 2, D2], FP, kind="ExternalInput")
    w2pm = nc.dram_tensor("w2pm", [D, 2, D2], FP, kind="ExternalInput")
    b1pm = nc.dram_tensor("b1pm", [D, 2], FP, kind="ExternalInput")
    b2pm = nc.dram_tensor("b2pm", [D, 2], FP, kind="ExternalInput")
    owpm = nc.dram_tensor("owpm", [D, 2], FP, kind="ExternalInput")
    out = nc.dram_tensor("out", [1, BC], FP, kind="ExternalOutput")

    mult = mybir.AluOpType.mult
    add = mybir.AluOpType.add
    amax = mybir.AluOpType.max

    with tile.TileContext(nc) as tc:
        with (
            tc.tile_pool(name="const", bufs=1) as constp,
            tc.tile_pool(name="gather", bufs=GBUFS) as gatherp,
            tc.tile_pool(name="prod", bufs=2) as prodp,
            tc.tile_pool(name="sm", bufs=2) as smp,
            tc.tile_pool(name="act", bufs=2) as actp,
            tc.tile_pool(name="hu", bufs=2) as hup,
            tc.tile_pool(name="pbc", bufs=2, space="PSUM") as pbcp,
            tc.tile_pool(name="phx", bufs=2, space="PSUM") as phxp,
            tc.tile_pool(name="pfc", bufs=2, space="PSUM") as pfcp,
        ):
            # ---- one-time loads (HWDGE) ----
            idxt_sb = constp.tile([LP, BC], I32)
            nc.sync.dma_start(idxt_sb[:], idxt.ap())
            xuf_sb = constp.tile([1, BC], I32)
            nc.sync.dma_start(xuf_sb[:], xuf.ap())
            mb_sb = constp.tile([NB, NG * LP], FP)
            nc.sync.dma_start(mb_sb[:], mbias.ap())
            xupm_sb = constp.tile([NB, NG], I32)
            nc.sync.dma_start(xupm_sb[:], xupm.ap())
            xbpm_sb = constp.tile([NB, NG], I32)
            nc.sync.dma_start(xbpm_sb[:], xbpm.ap())
            w1_sb = constp.tile([D, 2, D2], FP)
            nc.sync.dma_start(w1_sb[:], w1pm.ap())
            w2_sb = constp.tile([D, 2, D2], FP)
            nc.sync.dma_start(w2_sb[:], w2pm.ap())
            b1_sb = constp.tile([D, 2], FP)
            nc.sync.dma_start(b1_sb[:], b1pm.ap())
            b2_sb = constp.tile([D, 2], FP)
            nc.sync.dma_start(b2_sb[:], b2pm.ap())
            ow_sb = constp.tile([D, 2], FP)
            nc.sync.dma_start(ow_sb[:], owpm.ap())
            ones_sb = constp.tile([1, D], FP)
            nc.vector.memset(ones_sb[:], 1.0)
            out_acc = constp.tile([1, BC], FP)

            # all h_u / h_b rows, one row per (partition=in-group idx,
            # block=group): hur_all[p, g*D:(g+1)*D] = emb_u[x_u[g*NB+p]]
            # (the DGE fetches exactly one index per partition per call)
            hur_all = constp.tile([NB, NG * D], FP)
            hbr_all = constp.tile([NB, NG * D], FP)
            for g0 in range(NG):
                d0 = slice(g0 * D, (g0 + 1) * D)
                nc.gpsimd.indirect_dma_start(
                    out=hur_all[:, d0], out_offset=None, in_=embu.ap(),
                    in_offset=IndirectOffsetOnAxis(
                        ap=xupm_sb[:, g0:g0 + 1], axis=0),
                )
                nc.gpsimd.indirect_dma_start(
                    out=hbr_all[:, d0], out_offset=None, in_=embb.ap(),
                    in_offset=IndirectOffsetOnAxis(
                        ap=xbpm_sb[:, g0:g0 + 1], axis=0),
                )

            for g in range(NG):
                gsl = slice(g * NB, (g + 1) * NB)
                dsl = slice(g * D, (g + 1) * D)

                # ---- gather item rows: one [128,1]-index call per bundle ----
                gt = gatherp.tile([LP, NB * D2], BF)
                if g < GBUFS:
                    # first use of each buffer: clear stale/NaN SBUF so
                    # skipped (masked) rows stay finite
                    nc.vector.memset(gt[:], 0.0)
                for j in range(NB):
                    nc.gpsimd.indirect_dma_start(
                        out=gt[:, j * D2:(j + 1) * D2],
                        out_offset=None,
                        in_=tab.ap(),
                        in_offset=IndirectOffsetOnAxis(
                            ap=idxt_sb[:, g * NB + j:g * NB + j + 1], axis=0),
                        bounds_check=N_ITEM - 1,
                        oob_is_err=False,
                    )

                # ---- flat copy of the group's h_u rows onto partition 0
                # (PE rhs must start at partition 0/32/64, so collapse the
                # 32 per-partition rows into one 4KB row via SBUF->SBUF DMA)
                huf = hup.tile([1, NB * D], FP, tag="huf")
                nc.sync.dma_start(huf[:], hur_all[:, dsl])

                # ---- scores: per-bundle dot(A-half, h_u[j]) ----
                scores_g = smp.tile([LP, NB], FP, tag="scores")
                for q in range(NB // 4):
                    pbc = pbcp.tile([LP, 4 * D], FP, space="PSUM")
                    nc.tensor.matmul(
                        pbc[:], lhsT=ones_sb[:],
                        rhs=huf[:, q * 4 * D:(q + 1) * 4 * D],
                        start=True, stop=True,
                    )
                    for jj in range(4):
                        j = q * 4 + jj
                        prod = prodp.tile([LP, D], FP, tag="prod")
                        nc.vector.tensor_tensor(
                            out=prod[:],
                            in0=gt[:, j * D2 + D:(j + 1) * D2],
                            in1=pbc[:, jj * D:(jj + 1) * D],
                            op=mult,
                        )
                        nc.vector.reduce_sum(
                            out=scores_g[:, j:j + 1], in_=prod[:],
                            axis=mybir.AxisListType.X,
                        )

                # ---- softmax over items (transpose to bundles-on-partitions) ----
                scT = smp.tile([NB, LP], FP, tag="scT")
                for r in range(4):
                    nc.vector.transpose(
                        out=scT[0:32, r * 32:(r + 1) * 32],
                        in_=scores_g[r * 32:(r + 1) * 32, 0:32],
                    )
                sadj = smp.tile([NB, LP], FP, tag="sadj")
                mx = smp.tile([NB, 1], FP, tag="mx")
                nc.vector.tensor_tensor(
                    out=sadj[:],
                    in0=scT[:],
                    in1=mb_sb[:, g * LP:(g + 1) * LP],
                    op=add,
                )
                nc.vector.reduce_max(
                    out=mx[:], in_=sadj[:], axis=mybir.AxisListType.X,
                )
                nmx = smp.tile([NB, 1], FP, tag="nmx")
                nc.scalar.mul(nmx[:], mx[:], -1.0)
                ex = smp.tile([NB, LP], FP, tag="ex")
                zs = smp.tile([NB, 1], FP, tag="zs")
                nc.scalar.activation(
                    ex[:], sadj[:], mybir.ActivationFunctionType.Exp,
                    bias=nmx[:], scale=1.0, accum_out=zs[:],
                )
                rz = smp.tile([NB, 1], FP, tag="rz")
                nc.vector.reciprocal(rz[:], zs[:])
                wT = smp.tile([NB, LP], BF, tag="wT")
                nc.vector.tensor_scalar_mul(wT[:], ex[:], rz[:])
                w_g = smp.tile([LP, NB], BF, tag="w_g")
                for r in range(4):
                    nc.vector.transpose(
                        out=w_g[r * 32:(r + 1) * 32, 0:32],
                        in_=wT[0:32, r * 32:(r + 1) * 32],
                    )

                # ---- h_x.T columns via PE: item_e_j.T @ w_j ----
                phx = phxp.tile([D, NB], FP, space="PSUM")
                for j in range(NB):
                    nc.tensor.matmul(
                        phx[:, j:j + 1],
                        lhsT=gt[:, j * D2:j * D2 + D],
                        rhs=w_g[:, j:j + 1],
                        start=True, stop=True,
                    )

                # ---- transposes of h_u, h_b into [d, nb] ----
                huT = actp.tile([D, NB], FP, tag="huT")
                hbT = actp.tile([D, NB], FP, tag="hbT")
                for r in range(4):
                    nc.vector.transpose(
                        out=huT[r * 32:(r + 1) * 32, 0:32],
                        in_=hur_all[0:32, g * D + r * 32:g * D + (r + 1) * 32],
                    )
                    nc.vector.transpose(
                        out=hbT[r * 32:(r + 1) * 32, 0:32],
                        in_=hbr_all[0:32, g * D + r * 32:g * D + (r + 1) * 32],
                    )
                hbot = actp.tile([D, NB], FP, tag="hbot")
                nc.vector.tensor_add(hbot[:], phx[:], hbT[:])

                # ---- FC stack on [256, 32] activations ----
                def fc(w_sb, b_sb, rhs0, rhs1, tag):
                    outs = []
                    for mo in range(2):
                        pfc = pfcp.tile([D, NB], FP, space="PSUM", tag="pfc")
                        msl = slice(mo * D, (mo + 1) * D)
                        nc.tensor.matmul(pfc[:], lhsT=w_sb[:, 0, msl],
                                         rhs=rhs0[:], start=True, stop=False)
                        nc.tensor.matmul(pfc[:], lhsT=w_sb[:, 1, msl],
                                         rhs=rhs1[:], start=False, stop=True)
                        # leaky relu: max(x + b, alpha * (x + b))
                        xb = actp.tile([D, NB], FP, tag=f"{tag}xb{mo}")
                        nc.vector.tensor_scalar_add(xb[:], pfc[:],
                                                    b_sb[:, mo:mo + 1])
                        xs = actp.tile([D, NB], FP, tag=f"{tag}xs{mo}")
                        nc.vector.tensor_scalar_mul(xs[:], xb[:], alpha)
                        ao = actp.tile([D, NB], FP, tag=f"{tag}a{mo}")
                        nc.vector.tensor_tensor(out=ao[:], in0=xb[:],
                                                in1=xs[:], op=amax)
                        outs.append(ao)
                    return outs

                a1 = fc(w1_sb, b1_sb, huT, hbot, "f1")
                a2 = fc(w2_sb, b2_sb, a1[0], a1[1], "f2")

                po = phxp.tile([1, NB], FP, space="PSUM", tag="po")
                nc.tensor.matmul(po[:], lhsT=ow_sb[:, 0:1], rhs=a2[0][:],
                                 start=True, stop=False)
                nc.tensor.matmul(po[:], lhsT=ow_sb[:, 1:2], rhs=a2[1][:],
                                 start=False, stop=True)
                nc.scalar.activation(out_acc[:, gsl], po[:],
                                     mybir.ActivationFunctionType.Copy,
                                     bias=out_b0, scale=1.0)

            nc.sync.dma_start(out.ap(), out_acc[:])

    nc.compile()
    return nc


def _prep_inputs(x_u, x_b, items, mask, emb_u, emb_i, emb_b, A,
                 fc1_w, fc1_b, fc2_w, fc2_b, out_w, out_b):
    """Host-side packing: merged table, transposed/sentineled indices,
    per-core shards."""
    x_u = np.asarray(x_u).astype(np.int32)
    x_b = np.asarray(x_b).astype(np.int32)
    items = np.asarray(items).astype(np.int32)
    mask = np.asarray(mask).astype(bool)
    emb_u = np.ascontiguousarray(np.asarray(emb_u, dtype=np.float32))
    emb_i = np.asarray(emb_i, dtype=np.float32)
    emb_b = np.ascontiguousarray(np.asarray(emb_b, dtype=np.float32))
    A = np.asarray(A, dtype=np.float32)
    fc1_w = np.asarray(fc1_w, dtype=np.float32)
    fc2_w = np.asarray(fc2_w, dtype=np.float32)
    out_w = np.asarray(out_w, dtype=np.float32)
    fc1_b = np.asarray(fc1_b, dtype=np.float32)
    fc2_b = np.asarray(fc2_b, dtype=np.float32)

    tab = np.ascontiguousarray(
        np.concatenate([emb_i, A], axis=1).astype(BF_NP))  # [N_ITEM, 256]

    items_s = np.where(mask, items, SENTINEL)          # [B, L]
    w1pm = np.ascontiguousarray(
        fc1_w.T.reshape(2, D, D2).transpose(1, 0, 2))
    w2pm = np.ascontiguousarray(
        fc2_w.T.reshape(2, D, D2).transpose(1, 0, 2))
    b1pm = np.ascontiguousarray(fc1_b.reshape(2, D).T)
    b2pm = np.ascontiguousarray(fc2_b.reshape(2, D).T)
    owpm = np.ascontiguousarray(out_w.reshape(2, D).T)

    in_maps = []
    for c in range(NCORES):
        bsl = slice(c * BC, (c + 1) * BC)
        it_c = items_s[bsl]                            # [1024, 100]
        idxt = np.full((LP, BC), SENTINEL, np.int32)
        idxt[:L, :] = it_c.T
        mk = mask[bsl]                                 # [1024, 100]
        mb = np.full((BC, LP), -1.0e30, np.float32)
        mb[:, :L][mk] = 0.0
        # [BC, LP] -> [NB, NG*LP] with bundle (g*NB+p) -> partition p, block g
        mbias = np.ascontiguousarray(
            mb.reshape(NG, NB, LP).transpose(1, 0, 2).reshape(NB, NG * LP))
        xu_c = x_u[bsl]
        xb_c = x_b[bsl]
        in_maps.append({
            "tab": tab,
            "embu": emb_u,
            "embb": emb_b,
            "idxt": np.ascontiguousarray(idxt),
            "mbias": mbias,
            "xuf": np.ascontiguousarray(xu_c.reshape(1, BC)),
            "xupm": np.ascontiguousarray(xu_c.reshape(NG, NB).T),
            "xbpm": np.ascontiguousarray(xb_c.reshape(NG, NB).T),
            "w1pm": w1pm,
            "w2pm": w2pm,
            "b1pm": b1pm,
            "b2pm": b2pm,
            "owpm": owpm,
        })
    return in_maps


def kernel(x_u, x_b, items, mask, emb_u, emb_i, emb_b, A,
           fc1_w, fc1_b, fc2_w, fc2_b, out_w, out_b):
    global last_results
    out_b0 = float(np.asarray(out_b, dtype=np.float32).reshape(-1)[0])
    key = ("prog", out_b0)
    if key not in _cache:
        _cache[key] = _build_program(alpha=0.01, out_b0=out_b0)
    nc = _cache[key]

    in_maps = _prep_inputs(x_u, x_b, items, mask, emb_u, emb_i, emb_b, A,
                           fc1_w, fc1_b, fc2_w, fc2_b, out_w, out_b)

    res = run_bass_kernel_spmd(
        nc, in_maps, core_ids=list(range(NCORES)),
        trace=bool(int(os.environ.get("DAM_TRACE", "0"))),
    )
    last_results = res
    outs = [res.results[c]["out"].reshape(BC, 1) for c in range(NCORES)]
    return np.concatenate(outs, axis=0).astype(np.float32)

